# revision 3
# baseline (speedup 1.0000x reference)
"""KANLinear Trainium2 kernel.

Math (reference):
    xc     = clip(x, -1, 1)                                  # (N, in)
    base   = silu(xc) @ scale_base.T                         # (N, out)
    b0=1, b1=xc, b_k = 2*xc*b_{k-1} - 1
    spline[n,o] = sum_{i,k} scale_spline[o,i]*coeff[o,i,k]*b_k(xc[n,i])
    out    = base + spline + sum_i base_bias[o,i]

Device formulation: one big matmul over 8 features per input channel.
With t = clip(2x, -2, 2) and the shifted basis s_k = b_k + 1 (k>=2):
    s_2 = (t*0.5)*t,  s_{k+1} = (s_k - 1)*t     # one fused DVE op each
    features = [silu(t/2), t, s_2..s_7]         # 8 per input channel
    out[n,o] = sum_{i,f} F[f,i,n] * W[(f,i), o] + bias[o]
where W folds scale_base / scale_spline*coeff (and the s-shift and the
t=2x scaling), bias folds the k=0 term, the s-shift and base_bias.

Sharding: data-parallel over the 8192 tokens -> 1024 tokens per core
(core b gets batch b).  Each core computes its full [1024, 512] output
block; no collectives.  Host does layout transforms + the tiny bias add.
"""

import os

import numpy as np
import ml_dtypes

import concourse.bass as bass
import concourse.tile as tile
from concourse import bacc, mybir
from concourse import bass_utils

B, S, IN_F, OUT_F, K = 8, 1024, 512, 512, 8
NCORES = 8
N_PER = (B * S) // NCORES          # 1024 tokens per core
NTILES = N_PER // 128              # 8 psum accumulators
ICHUNKS = IN_F // 128              # 4 input-channel chunks
NFEAT = 8                          # silu, t, s2..s7
CHUNKS = ICHUNKS * NFEAT           # 32 contraction chunks of 128

ALU = mybir.AluOpType
ACT_FN = mybir.ActivationFunctionType

# "bf16" or "f32r": matmul operand precision
MM_DTYPE = os.environ.get("KERNEL_MM_DTYPE", "bf16")

_compiled = {}


def _build(mm_dtype: str):
    dt_feat = mybir.dt.bfloat16 if mm_dtype == "bf16" else mybir.dt.float32r
    nc = bacc.Bacc(
        "TRN2", target_bir_lowering=False, debug=False, enable_asserts=False
    )
    t_in = nc.dram_tensor(
        "t_in", [IN_F, N_PER], mybir.dt.float32, kind="ExternalInput"
    ).ap()
    w_in = nc.dram_tensor(
        "w_in", [CHUNKS * 128, OUT_F], dt_feat, kind="ExternalInput"
    ).ap()
    out = nc.dram_tensor(
        "out", [N_PER, OUT_F], mybir.dt.float32, kind="ExternalOutput"
    ).ap()

    with tile.TileContext(nc) as tc:
        with (
            tc.tile_pool(name="xp", bufs=2) as xp,
            tc.tile_pool(name="tp", bufs=2) as tp,
            tc.tile_pool(name="fp", bufs=4) as fp,
            tc.tile_pool(name="wp", bufs=8) as wp,
            tc.tile_pool(name="pp", bufs=1, space="PSUM") as pp,
        ):
            psums = [
                pp.tile([128, OUT_F], mybir.dt.float32, tag=f"ps{n}", name=f"ps{n}")
                for n in range(NTILES)
            ]
            # stream all weight chunks (each is read exactly once)
            wts = []
            for ch in range(CHUNKS):
                wt = wp.tile([128, OUT_F], dt_feat, tag="w", name=f"w{ch}")
                nc.sync.dma_start(out=wt, in_=w_in[ch * 128:(ch + 1) * 128, :])
                wts.append(wt)

            def mm_all(feat, ch):
                for n in range(NTILES):
                    nc.tensor.matmul(
                        psums[n][:, :],
                        feat[:, n * 128:(n + 1) * 128],
                        wts[ch][:, :],
                        start=(ch == 0),
                        stop=(ch == CHUNKS - 1),
                    )

            for c in range(ICHUNKS):
                xt = xp.tile([128, N_PER], mybir.dt.float32, tag="x", name=f"x{c}")
                nc.sync.dma_start(out=xt, in_=t_in[c * 128:(c + 1) * 128, :])
                # t = clip(2x, -2, 2)   (host sends 2x)
                t = tp.tile([128, N_PER], dt_feat, tag="t", name=f"t{c}")
                nc.vector.tensor_scalar(
                    out=t, in0=xt, scalar1=2.0, scalar2=-2.0,
                    op0=ALU.min, op1=ALU.max,
                )
                # f0 = silu(t/2) = sigmoid(t/2) * (t/2)
                sg = fp.tile([128, N_PER], dt_feat, tag="f", name=f"sg{c}")
                nc.scalar.activation(out=sg, in_=t, func=ACT_FN.Sigmoid, scale=0.5)
                f0 = fp.tile([128, N_PER], dt_feat, tag="f", name=f"silu{c}")
                nc.vector.scalar_tensor_tensor(
                    out=f0, in0=t, scalar=0.5, in1=sg, op0=ALU.mult, op1=ALU.mult
                )
                mm_all(f0, c * NFEAT + 0)
                # f1 = t
                mm_all(t, c * NFEAT + 1)
                # s2 = (t*0.5)*t ;  s_{k+1} = (s_k - 1)*t
                prev = fp.tile([128, N_PER], dt_feat, tag="f", name=f"s2_{c}")
                nc.vector.scalar_tensor_tensor(
                    out=prev, in0=t, scalar=0.5, in1=t, op0=ALU.mult, op1=ALU.mult
                )
                mm_all(prev, c * NFEAT + 2)
                for k in range(3, NFEAT):
                    s = fp.tile([128, N_PER], dt_feat, tag="f", name=f"s{k}_{c}")
                    nc.vector.scalar_tensor_tensor(
                        out=s, in0=prev, scalar=-1.0, in1=t,
                        op0=ALU.add, op1=ALU.mult,
                    )
                    mm_all(s, c * NFEAT + k)
                    prev = s

            with tc.tile_pool(name="op", bufs=2) as op:
                for n in range(NTILES):
                    ot = op.tile([128, OUT_F], mybir.dt.float32, tag="o",
                                 name=f"o{n}")
                    nc.vector.tensor_copy(out=ot, in_=psums[n][:, :])
                    nc.sync.dma_start(
                        out=out[n * 128:(n + 1) * 128, :], in_=ot
                    )

    nc.compile()
    return nc


def _get_nc(mm_dtype: str):
    if mm_dtype not in _compiled:
        _compiled[mm_dtype] = _build(mm_dtype)
    return _compiled[mm_dtype]


def _prep_weights(coeff, scale_base, scale_spline, base_bias, mm_dtype: str):
    """Fold scales/basis-shift into one [(f,i) x out] matrix + bias vector."""
    w_spl = (scale_spline.astype(np.float64)[:, :, None]
             * coeff.astype(np.float64))                      # (o, i, k)
    W = np.empty((ICHUNKS, NFEAT, 128, OUT_F), np.float64)
    for c in range(ICHUNKS):
        sl = slice(c * 128, (c + 1) * 128)
        W[c, 0] = scale_base.astype(np.float64).T[sl]         # silu(t/2)=silu(xc)
        W[c, 1] = w_spl[:, sl, 1].T * 0.5                     # feature t = 2*xc
        for k in range(2, NFEAT):
            W[c, k] = w_spl[:, sl, k].T                       # feature s_k=b_k+1
    # bias: k=0 term (b0=1), minus the +1 shift of s_2..s_7, plus base_bias
    bias = (w_spl[:, :, 0] - w_spl[:, :, 2:].sum(-1)).sum(1) \
        + base_bias.astype(np.float64).sum(1)
    np_dt = ml_dtypes.bfloat16 if mm_dtype == "bf16" else np.float32
    return (np.ascontiguousarray(W.reshape(CHUNKS * 128, OUT_F)).astype(np_dt),
            bias.astype(np.float32))


def kernel(x, coeff, scale_base, scale_spline, base_bias):
    mm_dtype = MM_DTYPE
    nc = _get_nc(mm_dtype)
    W_dev, bias = _prep_weights(coeff, scale_base, scale_spline, base_bias,
                                mm_dtype)
    xr = x.reshape(NCORES, N_PER, IN_F)
    in_maps = []
    for b in range(NCORES):
        t_b = np.ascontiguousarray((2.0 * xr[b]).T.astype(np.float32))
        in_maps.append({"t_in": t_b, "w_in": W_dev})

    trace = bool(int(os.environ.get("KERNEL_TRACE", "0")))
    res = bass_utils.run_bass_kernel_spmd(
        nc, in_maps, core_ids=list(range(NCORES)), trace=trace
    )
    global LAST_RESULT
    LAST_RESULT = res
    out = np.stack([res.results[b]["out"] for b in range(NCORES)], axis=0)
    out = out + bias[None, None, :]
    return out.reshape(B, S, OUT_F).astype(np.float32)


LAST_RESULT = None


# revision 6
# speedup vs baseline: 63.2910x; 63.2910x over previous
"""KANLinear Trainium2 kernel.

Math (reference):
    xc     = clip(x, -1, 1)                                  # (N, in)
    base   = silu(xc) @ scale_base.T                         # (N, out)
    b0=1, b1=xc, b_k = 2*xc*b_{k-1} - 1
    spline[n,o] = sum_{i,k} scale_spline[o,i]*coeff[o,i,k]*b_k(xc[n,i])
    out    = base + spline + sum_i base_bias[o,i]

Device formulation: one big matmul over 8 features per input channel.
With t = clip(2x, -2, 2) and the shifted basis s_k = b_k + 1 (k>=2):
    s_2 = (t*0.5)*t,  s_{k+1} = (s_k - 1)*t     # one fused DVE op each
    features = [silu(t/2), t, s_2..s_7]         # 8 per input channel
    out[n,o] = sum_{i,f} F[f,i,n] * W[(f,i), o] + bias[o]
where W folds scale_base / scale_spline*coeff (and the s-shift and the
t=2x scaling), bias folds the k=0 term, the s-shift and base_bias.

Sharding: data-parallel over the 8192 tokens -> 1024 tokens per core
(core b gets batch b).  Each core computes its full [1024, 512] output
block; no collectives.  Host does layout transforms + the tiny bias add.
"""

import os

import numpy as np
import ml_dtypes

import concourse.bass as bass
import concourse.tile as tile
from concourse import bacc, mybir
from concourse import bass_utils

B, S, IN_F, OUT_F, K = 8, 1024, 512, 512, 8
NCORES = 8
N_PER = (B * S) // NCORES          # 1024 tokens per core
NTILES = N_PER // 128              # 8 psum accumulators
ICHUNKS = IN_F // 128              # 4 input-channel chunks
NFEAT = 8                          # silu, t, s2..s7
CHUNKS = ICHUNKS * NFEAT           # 32 contraction chunks of 128

ALU = mybir.AluOpType
ACT_FN = mybir.ActivationFunctionType

# "bf16" or "f32r": matmul operand precision
MM_DTYPE = os.environ.get("KERNEL_MM_DTYPE", "bf16")

_compiled = {}


def _build(mm_dtype: str, repeats: int = 1):
    dt_feat = mybir.dt.bfloat16 if mm_dtype == "bf16" else mybir.dt.float32r
    nc = bacc.Bacc(
        "TRN2", target_bir_lowering=False, debug=False, enable_asserts=False
    )
    t_in = nc.dram_tensor(
        "t_in", [IN_F, N_PER], mybir.dt.float32, kind="ExternalInput"
    ).ap()
    w_in = nc.dram_tensor(
        "w_in", [CHUNKS * 128, OUT_F], dt_feat, kind="ExternalInput"
    ).ap()
    out = nc.dram_tensor(
        "out", [N_PER, OUT_F], mybir.dt.float32, kind="ExternalOutput"
    ).ap()

    with tile.TileContext(nc) as tc:
        with (
            tc.tile_pool(name="xp", bufs=2) as xp,
            tc.tile_pool(name="tp", bufs=2) as tp,
            tc.tile_pool(name="fp", bufs=4) as fp,
            tc.tile_pool(name="wp", bufs=8) as wp,
            tc.tile_pool(name="op", bufs=2) as op,
            tc.tile_pool(name="pp", bufs=1, space="PSUM") as pp,
        ):
            for rep in range(repeats):
                psums = [
                    pp.tile([128, OUT_F], mybir.dt.float32, tag=f"ps{n}",
                            name=f"ps{n}_{rep}")
                    for n in range(NTILES)
                ]
                # stream weight chunks (each read exactly once); interleave
                # the x-chunk loads so the first activations don't queue
                # behind 4 MB of weight DMA
                xts = []
                wts = []

                def emit_x(c, rep=rep, xts=xts):
                    xt = xp.tile([128, N_PER], mybir.dt.float32, tag="x",
                                 name=f"x{c}_{rep}")
                    nc.sync.dma_start(out=xt, in_=t_in[c * 128:(c + 1) * 128, :])
                    xts.append(xt)

                emit_x(0)
                for ch in range(CHUNKS):
                    wt = wp.tile([128, OUT_F], dt_feat, tag="w",
                                 name=f"w{ch}_{rep}")
                    nc.sync.dma_start(out=wt,
                                      in_=w_in[ch * 128:(ch + 1) * 128, :])
                    wts.append(wt)
                    if ch % 8 == 7 and ch // 8 + 1 < ICHUNKS:
                        emit_x(ch // 8 + 1)

                def mm_all(feat, ch, psums=psums, wts=wts):
                    for n in range(NTILES):
                        nc.tensor.matmul(
                            psums[n][:, :],
                            feat[:, n * 128:(n + 1) * 128],
                            wts[ch][:, :],
                            start=(ch == 0),
                            stop=(ch == CHUNKS - 1),
                        )

                for c in range(ICHUNKS):
                    xt = xts[c]
                    # t = clip(2x, -2, 2)   (host sends 2x)
                    t = tp.tile([128, N_PER], dt_feat, tag="t", name=f"t{c}_{rep}")
                    nc.vector.tensor_scalar(
                        out=t, in0=xt, scalar1=2.0, scalar2=-2.0,
                        op0=ALU.min, op1=ALU.max,
                    )
                    # f0 = silu(t/2) = sigmoid(t/2) * (t/2)
                    sg = fp.tile([128, N_PER], dt_feat, tag="f", name=f"sg{c}_{rep}")
                    nc.scalar.activation(out=sg, in_=t, func=ACT_FN.Sigmoid,
                                         scale=0.5)
                    f0 = fp.tile([128, N_PER], dt_feat, tag="f",
                                 name=f"silu{c}_{rep}")
                    nc.vector.scalar_tensor_tensor(
                        out=f0, in0=t, scalar=0.5, in1=sg,
                        op0=ALU.mult, op1=ALU.mult
                    )
                    mm_all(f0, c * NFEAT + 0)
                    # f1 = t
                    mm_all(t, c * NFEAT + 1)
                    # s2 = (t*0.5)*t ;  s_{k+1} = (s_k - 1)*t
                    prev = fp.tile([128, N_PER], dt_feat, tag="f",
                                   name=f"s2_{c}_{rep}")
                    nc.vector.scalar_tensor_tensor(
                        out=prev, in0=t, scalar=0.5, in1=t,
                        op0=ALU.mult, op1=ALU.mult
                    )
                    mm_all(prev, c * NFEAT + 2)
                    for k in range(3, NFEAT):
                        s = fp.tile([128, N_PER], dt_feat, tag="f",
                                    name=f"s{k}_{c}_{rep}")
                        nc.vector.scalar_tensor_tensor(
                            out=s, in0=prev, scalar=-1.0, in1=t,
                            op0=ALU.add, op1=ALU.mult,
                        )
                        mm_all(s, c * NFEAT + k)
                        prev = s

                for n in range(NTILES):
                    ot = op.tile([128, OUT_F], mybir.dt.float32, tag="o",
                                 name=f"o{n}_{rep}")
                    nc.vector.tensor_copy(out=ot, in_=psums[n][:, :])
                    nc.sync.dma_start(
                        out=out[n * 128:(n + 1) * 128, :], in_=ot
                    )

    nc.compile()
    return nc


def _get_nc(mm_dtype: str, repeats: int = 1):
    key = (mm_dtype, repeats)
    if key not in _compiled:
        _compiled[key] = _build(mm_dtype, repeats)
    return _compiled[key]


def _prep_weights(coeff, scale_base, scale_spline, base_bias, mm_dtype: str):
    """Fold scales/basis-shift into one [(f,i) x out] matrix + bias vector."""
    w_spl = (scale_spline.astype(np.float64)[:, :, None]
             * coeff.astype(np.float64))                      # (o, i, k)
    W = np.empty((ICHUNKS, NFEAT, 128, OUT_F), np.float64)
    for c in range(ICHUNKS):
        sl = slice(c * 128, (c + 1) * 128)
        W[c, 0] = scale_base.astype(np.float64).T[sl]         # silu(t/2)=silu(xc)
        W[c, 1] = w_spl[:, sl, 1].T * 0.5                     # feature t = 2*xc
        for k in range(2, NFEAT):
            W[c, k] = w_spl[:, sl, k].T                       # feature s_k=b_k+1
    # bias: k=0 term (b0=1), minus the +1 shift of s_2..s_7, plus base_bias
    bias = (w_spl[:, :, 0] - w_spl[:, :, 2:].sum(-1)).sum(1) \
        + base_bias.astype(np.float64).sum(1)
    np_dt = ml_dtypes.bfloat16 if mm_dtype == "bf16" else np.float32
    return (np.ascontiguousarray(W.reshape(CHUNKS * 128, OUT_F)).astype(np_dt),
            bias.astype(np.float32))


def kernel(x, coeff, scale_base, scale_spline, base_bias):
    mm_dtype = MM_DTYPE
    nc = _get_nc(mm_dtype)
    W_dev, bias = _prep_weights(coeff, scale_base, scale_spline, base_bias,
                                mm_dtype)
    xr = x.reshape(NCORES, N_PER, IN_F)
    in_maps = []
    for b in range(NCORES):
        t_b = np.ascontiguousarray((2.0 * xr[b]).T.astype(np.float32))
        in_maps.append({"t_in": t_b, "w_in": W_dev})

    trace = bool(int(os.environ.get("KERNEL_TRACE", "0")))
    res = bass_utils.run_bass_kernel_spmd(
        nc, in_maps, core_ids=list(range(NCORES)), trace=trace
    )
    global LAST_RESULT
    LAST_RESULT = res
    out = np.stack([res.results[b]["out"] for b in range(NCORES)], axis=0)
    out = out + bias[None, None, :]
    return out.reshape(B, S, OUT_F).astype(np.float32)


LAST_RESULT = None


# revision 12
# speedup vs baseline: 137.7226x; 2.1760x over previous
"""KANLinear Trainium2 kernel.

Math (reference):
    xc     = clip(x, -1, 1)                                  # (N, in)
    base   = silu(xc) @ scale_base.T                         # (N, out)
    b0=1, b1=xc, b_k = 2*xc*b_{k-1} - 1
    spline[n,o] = sum_{i,k} scale_spline[o,i]*coeff[o,i,k]*b_k(xc[n,i])
    out    = base + spline + sum_i base_bias[o,i]

Device formulation: one big matmul over 8 features per input channel.
With t = clip(2x, -2, 2) and the shifted basis s_k = b_k + 1 (k>=2):
    s_2 = (t*0.5)*t,  s_{k+1} = (s_k - 1)*t     # one fused DVE op each
    features = [silu(t/2), t, s_2..s_7]         # 8 per input channel
    out[n,o] = sum_{i,f} F[f,i,n] * W[(f,i), o] + bias[o]
where W folds scale_base / scale_spline*coeff (and the s-shift and the
t=2x scaling), bias folds the k=0 term, the s-shift and base_bias.

Sharding: data-parallel over the 8192 tokens -> 1024 tokens per core
(core b gets batch b).  Each core computes its full [1024, 512] output
block; no collectives.  Host does layout transforms + the tiny bias add.
"""

import os

import numpy as np
import ml_dtypes

import concourse.bass as bass
import concourse.tile as tile
from concourse import bacc, mybir
from concourse import bass_utils

B, S, IN_F, OUT_F, K = 8, 1024, 512, 512, 8
NCORES = 8
N_PER = (B * S) // NCORES          # 1024 tokens per core
NTILES = N_PER // 128              # 8 psum accumulators
ICHUNKS = IN_F // 128              # 4 input-channel chunks
NFEAT = 8                          # silu, t, s2..s7
CHUNKS = ICHUNKS * NFEAT           # 32 contraction chunks of 128

ALU = mybir.AluOpType
ACT_FN = mybir.ActivationFunctionType

# "bf16" or "f32r": matmul operand precision
MM_DTYPE = os.environ.get("KERNEL_MM_DTYPE", "bf16")

_compiled = {}


def _build(mm_dtype: str, repeats: int = 1):
    dt_feat = mybir.dt.bfloat16 if mm_dtype == "bf16" else mybir.dt.float32r
    nc = bacc.Bacc(
        "TRN2", target_bir_lowering=False, debug=False, enable_asserts=False
    )
    t_in = nc.dram_tensor(
        "t_in", [IN_F, N_PER], mybir.dt.float32, kind="ExternalInput"
    ).ap()
    w_in = nc.dram_tensor(
        "w_in", [CHUNKS * 128, OUT_F], dt_feat, kind="ExternalInput"
    ).ap()
    # transposed output: [out_features, tokens]; host transposes back
    out = nc.dram_tensor(
        "out", [OUT_F, N_PER], mybir.dt.float32, kind="ExternalOutput"
    ).ap()

    with tile.TileContext(nc) as tc:
        with (
            tc.tile_pool(name="xp", bufs=2) as xp,
            tc.tile_pool(name="tp", bufs=2) as tp,
            tc.tile_pool(name="fp", bufs=4) as fp,
            tc.tile_pool(name="wp", bufs=8) as wp,
            tc.tile_pool(name="op", bufs=2) as op,
            tc.tile_pool(name="pp", bufs=1, space="PSUM") as pp,
        ):
            for rep in range(repeats):
                # psum[ot] holds out.T rows ot*128..+128: [128 o, 1024 tok]
                psums = [
                    pp.tile([128, N_PER], mybir.dt.float32, tag=f"ps{ot}",
                            name=f"ps{ot}_{rep}")
                    for ot in range(OUT_F // 128)
                ]
                # stream weight chunks (each read exactly once); interleave
                # the x-chunk loads so the first activations don't queue
                # behind 4 MB of weight DMA
                xts = []
                wts = []

                def emit_x(c, rep=rep, xts=xts):
                    xt = xp.tile([128, N_PER], mybir.dt.float32, tag="x",
                                 name=f"x{c}_{rep}")
                    nc.sync.dma_start(out=xt, in_=t_in[c * 128:(c + 1) * 128, :])
                    xts.append(xt)

                emit_x(0)
                for ch in range(CHUNKS):
                    wt = wp.tile([128, OUT_F], dt_feat, tag="w",
                                 name=f"w{ch}_{rep}")
                    nc.sync.dma_start(out=wt,
                                      in_=w_in[ch * 128:(ch + 1) * 128, :])
                    wts.append(wt)
                    if ch % 8 == 7 and ch // 8 + 1 < ICHUNKS:
                        emit_x(ch // 8 + 1)

                def mm_all(feat, ch, psums=psums, wts=wts):
                    # lhsT = W subtile (stationary, shared by both halves so
                    # LDWEIGHTS amortizes); rhs = features (moving, FD=512)
                    for ot in range(OUT_F // 128):
                        for h in range(N_PER // 512):
                            nc.tensor.matmul(
                                psums[ot][:, h * 512:(h + 1) * 512],
                                wts[ch][:, ot * 128:(ot + 1) * 128],
                                feat[:, h * 512:(h + 1) * 512],
                                start=(ch == 0),
                                stop=(ch == CHUNKS - 1),
                            )

                for c in range(ICHUNKS):
                    xt = xts[c]
                    # t = clip(2x, -2, 2)   (host sends 2x)
                    t = tp.tile([128, N_PER], dt_feat, tag="t", name=f"t{c}_{rep}")
                    nc.vector.tensor_scalar(
                        out=t, in0=xt, scalar1=2.0, scalar2=-2.0,
                        op0=ALU.min, op1=ALU.max,
                    )
                    # f0 = silu(t/2) = sigmoid(t/2) * (t/2)
                    sg = fp.tile([128, N_PER], dt_feat, tag="f", name=f"sg{c}_{rep}")
                    nc.scalar.activation(out=sg, in_=t, func=ACT_FN.Sigmoid,
                                         scale=0.5)
                    f0 = fp.tile([128, N_PER], dt_feat, tag="f",
                                 name=f"silu{c}_{rep}")
                    nc.vector.scalar_tensor_tensor(
                        out=f0, in0=t, scalar=0.5, in1=sg,
                        op0=ALU.mult, op1=ALU.mult
                    )
                    mm_all(f0, c * NFEAT + 0)
                    # f1 = t
                    mm_all(t, c * NFEAT + 1)
                    # s2 = (t*0.5)*t ;  s_{k+1} = (s_k - 1)*t
                    prev = fp.tile([128, N_PER], dt_feat, tag="f",
                                   name=f"s2_{c}_{rep}")
                    nc.vector.scalar_tensor_tensor(
                        out=prev, in0=t, scalar=0.5, in1=t,
                        op0=ALU.mult, op1=ALU.mult
                    )
                    mm_all(prev, c * NFEAT + 2)
                    for k in range(3, NFEAT):
                        s = fp.tile([128, N_PER], dt_feat, tag="f",
                                    name=f"s{k}_{c}_{rep}")
                        nc.vector.scalar_tensor_tensor(
                            out=s, in0=prev, scalar=-1.0, in1=t,
                            op0=ALU.add, op1=ALU.mult,
                        )
                        mm_all(s, c * NFEAT + k)
                        prev = s

                for ot_i in range(OUT_F // 128):
                    osb = op.tile([128, N_PER], mybir.dt.float32, tag="o",
                                  name=f"o{ot_i}_{rep}")
                    nc.vector.tensor_copy(out=osb, in_=psums[ot_i][:, :])
                    nc.sync.dma_start(
                        out=out[ot_i * 128:(ot_i + 1) * 128, :], in_=osb
                    )

    nc.compile()
    return nc


def _get_nc(mm_dtype: str, repeats: int = 1):
    key = (mm_dtype, repeats)
    if key not in _compiled:
        _compiled[key] = _build(mm_dtype, repeats)
    return _compiled[key]


def _prep_weights(coeff, scale_base, scale_spline, base_bias, mm_dtype: str):
    """Fold scales/basis-shift into one [(f,i) x out] matrix + bias vector."""
    w_spl = (scale_spline.astype(np.float64)[:, :, None]
             * coeff.astype(np.float64))                      # (o, i, k)
    W = np.empty((ICHUNKS, NFEAT, 128, OUT_F), np.float64)
    for c in range(ICHUNKS):
        sl = slice(c * 128, (c + 1) * 128)
        W[c, 0] = scale_base.astype(np.float64).T[sl]         # silu(t/2)=silu(xc)
        W[c, 1] = w_spl[:, sl, 1].T * 0.5                     # feature t = 2*xc
        for k in range(2, NFEAT):
            W[c, k] = w_spl[:, sl, k].T                       # feature s_k=b_k+1
    # bias: k=0 term (b0=1), minus the +1 shift of s_2..s_7, plus base_bias
    bias = (w_spl[:, :, 0] - w_spl[:, :, 2:].sum(-1)).sum(1) \
        + base_bias.astype(np.float64).sum(1)
    np_dt = ml_dtypes.bfloat16 if mm_dtype == "bf16" else np.float32
    return (np.ascontiguousarray(W.reshape(CHUNKS * 128, OUT_F)).astype(np_dt),
            bias.astype(np.float32))


def kernel(x, coeff, scale_base, scale_spline, base_bias):
    x = np.asarray(x, dtype=np.float32)
    coeff = np.asarray(coeff, dtype=np.float32)
    scale_base = np.asarray(scale_base, dtype=np.float32)
    scale_spline = np.asarray(scale_spline, dtype=np.float32)
    base_bias = np.asarray(base_bias, dtype=np.float32)
    mm_dtype = MM_DTYPE
    nc = _get_nc(mm_dtype)
    W_dev, bias = _prep_weights(coeff, scale_base, scale_spline, base_bias,
                                mm_dtype)
    xr = x.reshape(NCORES, N_PER, IN_F)
    in_maps = []
    for b in range(NCORES):
        t_b = np.ascontiguousarray((2.0 * xr[b]).T.astype(np.float32))
        in_maps.append({"t_in": t_b, "w_in": W_dev})

    trace = bool(int(os.environ.get("KERNEL_TRACE", "0")))
    res = bass_utils.run_bass_kernel_spmd(
        nc, in_maps, core_ids=list(range(NCORES)), trace=trace
    )
    global LAST_RESULT
    LAST_RESULT = res
    out = np.stack([res.results[b]["out"].T for b in range(NCORES)], axis=0)
    out = out + bias[None, None, :]
    return out.reshape(B, S, OUT_F).astype(np.float32)


LAST_RESULT = None


# revision 14
# speedup vs baseline: 164.4538x; 1.1941x over previous
"""KANLinear Trainium2 kernel.

Math (reference):
    xc     = clip(x, -1, 1)                                  # (N, in)
    base   = silu(xc) @ scale_base.T                         # (N, out)
    b0=1, b1=xc, b_k = 2*xc*b_{k-1} - 1
    spline[n,o] = sum_{i,k} scale_spline[o,i]*coeff[o,i,k]*b_k(xc[n,i])
    out    = base + spline + sum_i base_bias[o,i]

Device formulation: one big matmul over 8 features per input channel.
With t = clip(2x, -2, 2) and the shifted basis s_k = b_k + 1 (k>=2):
    s_2 = (t*0.5)*t,  s_{k+1} = (s_k - 1)*t     # one fused DVE op each
    features = [silu(t/2), t, s_2..s_7]         # 8 per input channel
    out[n,o] = sum_{i,f} F[f,i,n] * W[(f,i), o] + bias[o]
where W folds scale_base / scale_spline*coeff (and the s-shift and the
t=2x scaling), bias folds the k=0 term, the s-shift and base_bias.

Sharding: data-parallel over the 8192 tokens -> 1024 tokens per core
(core b gets batch b).  Each core computes its full [1024, 512] output
block; no collectives.  Host does layout transforms + the tiny bias add.
"""

import os

import numpy as np
import ml_dtypes

import concourse.bass as bass
import concourse.tile as tile
from concourse import bacc, mybir
from concourse import bass_utils

B, S, IN_F, OUT_F, K = 8, 1024, 512, 512, 8
NCORES = 8
N_PER = (B * S) // NCORES          # 1024 tokens per core
NTILES = N_PER // 128              # 8 psum accumulators
ICHUNKS = IN_F // 128              # 4 input-channel chunks
NFEAT = 8                          # silu, t, s2..s7
CHUNKS = ICHUNKS * NFEAT           # 32 contraction chunks of 128

ALU = mybir.AluOpType
ACT_FN = mybir.ActivationFunctionType

# "bf16" or "f32r": matmul operand precision
MM_DTYPE = os.environ.get("KERNEL_MM_DTYPE", "bf16")

_compiled = {}


def _build(mm_dtype: str, repeats: int = 1):
    dt_feat = mybir.dt.bfloat16 if mm_dtype == "bf16" else mybir.dt.float32r
    nc = bacc.Bacc(
        "TRN2", target_bir_lowering=False, debug=False, enable_asserts=False
    )
    t_in = nc.dram_tensor(
        "t_in", [IN_F, N_PER], mybir.dt.float32, kind="ExternalInput"
    ).ap()
    w_in = nc.dram_tensor(
        "w_in", [CHUNKS * 128, OUT_F], dt_feat, kind="ExternalInput"
    ).ap()
    # transposed output: [out_features, tokens]; host transposes back
    out = nc.dram_tensor(
        "out", [OUT_F, N_PER], mybir.dt.float32, kind="ExternalOutput"
    ).ap()

    with tile.TileContext(nc) as tc:
        with (
            tc.tile_pool(name="xp", bufs=2) as xp,
            tc.tile_pool(name="tp", bufs=2) as tp,
            tc.tile_pool(name="fp", bufs=4) as fp,
            tc.tile_pool(name="wp", bufs=8) as wp,
            tc.tile_pool(name="op", bufs=2) as op,
            tc.tile_pool(name="pp", bufs=1, space="PSUM") as pp,
        ):
            for rep in range(repeats):
                # psum[ot] holds out.T rows ot*128..+128: [128 o, 1024 tok]
                psums = [
                    pp.tile([128, N_PER], mybir.dt.float32, tag=f"ps{ot}",
                            name=f"ps{ot}_{rep}")
                    for ot in range(OUT_F // 128)
                ]
                # stream weight chunks (each read exactly once); interleave
                # the x-chunk loads so the first activations don't queue
                # behind 4 MB of weight DMA
                xts = []
                wts = []

                def emit_x(c, rep=rep, xts=xts):
                    xt = xp.tile([128, N_PER], mybir.dt.float32, tag="x",
                                 name=f"x{c}_{rep}")
                    nc.sync.dma_start(out=xt, in_=t_in[c * 128:(c + 1) * 128, :])
                    xts.append(xt)

                emit_x(0)
                for ch in range(CHUNKS):
                    wt = wp.tile([128, OUT_F], dt_feat, tag="w",
                                 name=f"w{ch}_{rep}")
                    nc.sync.dma_start(out=wt,
                                      in_=w_in[ch * 128:(ch + 1) * 128, :])
                    wts.append(wt)
                    if ch % 8 == 7 and ch // 8 + 1 < ICHUNKS:
                        emit_x(ch // 8 + 1)

                def mm_all(feat, ch, psums=psums, wts=wts):
                    # lhsT = W subtile (stationary, shared by both halves so
                    # LDWEIGHTS amortizes); rhs = features (moving, FD=512)
                    for ot in range(OUT_F // 128):
                        for h in range(N_PER // 512):
                            nc.tensor.matmul(
                                psums[ot][:, h * 512:(h + 1) * 512],
                                wts[ch][:, ot * 128:(ot + 1) * 128],
                                feat[:, h * 512:(h + 1) * 512],
                                start=(ch == 0),
                                stop=(ch == CHUNKS - 1),
                            )

                for c in range(ICHUNKS):
                    xt = xts[c]
                    # t = clip(2x, -2, 2)   (host sends 2x)
                    t = tp.tile([128, N_PER], dt_feat, tag="t", name=f"t{c}_{rep}")
                    nc.vector.tensor_scalar(
                        out=t, in0=xt, scalar1=2.0, scalar2=-2.0,
                        op0=ALU.min, op1=ALU.max,
                    )
                    # f0 = t (ready first: only the clip op precedes it)
                    mm_all(t, c * NFEAT + 0)
                    # f1 = silu(t/2) = sigmoid(t/2) * (t/2)
                    sg = fp.tile([128, N_PER], dt_feat, tag="f", name=f"sg{c}_{rep}")
                    nc.scalar.activation(out=sg, in_=t, func=ACT_FN.Sigmoid,
                                         scale=0.5)
                    f1 = fp.tile([128, N_PER], dt_feat, tag="f",
                                 name=f"silu{c}_{rep}")
                    nc.vector.scalar_tensor_tensor(
                        out=f1, in0=t, scalar=0.5, in1=sg,
                        op0=ALU.mult, op1=ALU.mult
                    )
                    mm_all(f1, c * NFEAT + 1)
                    # s2 = (t*0.5)*t ;  s_{k+1} = (s_k - 1)*t
                    prev = fp.tile([128, N_PER], dt_feat, tag="f",
                                   name=f"s2_{c}_{rep}")
                    nc.vector.scalar_tensor_tensor(
                        out=prev, in0=t, scalar=0.5, in1=t,
                        op0=ALU.mult, op1=ALU.mult
                    )
                    mm_all(prev, c * NFEAT + 2)
                    for k in range(3, NFEAT):
                        s = fp.tile([128, N_PER], dt_feat, tag="f",
                                    name=f"s{k}_{c}_{rep}")
                        nc.vector.scalar_tensor_tensor(
                            out=s, in0=prev, scalar=-1.0, in1=t,
                            op0=ALU.add, op1=ALU.mult,
                        )
                        mm_all(s, c * NFEAT + k)
                        prev = s

                for ot_i in range(OUT_F // 128):
                    osb = op.tile([128, N_PER], mybir.dt.float32, tag="o",
                                  name=f"o{ot_i}_{rep}")
                    nc.vector.tensor_copy(out=osb, in_=psums[ot_i][:, :])
                    nc.sync.dma_start(
                        out=out[ot_i * 128:(ot_i + 1) * 128, :], in_=osb
                    )

    nc.compile()
    return nc


def _get_nc(mm_dtype: str, repeats: int = 1):
    key = (mm_dtype, repeats)
    if key not in _compiled:
        _compiled[key] = _build(mm_dtype, repeats)
    return _compiled[key]


def _prep_weights(coeff, scale_base, scale_spline, base_bias, mm_dtype: str):
    """Fold scales/basis-shift into one [(f,i) x out] matrix + bias vector."""
    w_spl = (scale_spline.astype(np.float64)[:, :, None]
             * coeff.astype(np.float64))                      # (o, i, k)
    W = np.empty((ICHUNKS, NFEAT, 128, OUT_F), np.float64)
    for c in range(ICHUNKS):
        sl = slice(c * 128, (c + 1) * 128)
        W[c, 0] = w_spl[:, sl, 1].T * 0.5                     # feature t = 2*xc
        W[c, 1] = scale_base.astype(np.float64).T[sl]         # silu(t/2)=silu(xc)
        for k in range(2, NFEAT):
            W[c, k] = w_spl[:, sl, k].T                       # feature s_k=b_k+1
    # bias: k=0 term (b0=1), minus the +1 shift of s_2..s_7, plus base_bias
    bias = (w_spl[:, :, 0] - w_spl[:, :, 2:].sum(-1)).sum(1) \
        + base_bias.astype(np.float64).sum(1)
    np_dt = ml_dtypes.bfloat16 if mm_dtype == "bf16" else np.float32
    return (np.ascontiguousarray(W.reshape(CHUNKS * 128, OUT_F)).astype(np_dt),
            bias.astype(np.float32))


def kernel(x, coeff, scale_base, scale_spline, base_bias):
    x = np.asarray(x, dtype=np.float32)
    coeff = np.asarray(coeff, dtype=np.float32)
    scale_base = np.asarray(scale_base, dtype=np.float32)
    scale_spline = np.asarray(scale_spline, dtype=np.float32)
    base_bias = np.asarray(base_bias, dtype=np.float32)
    mm_dtype = MM_DTYPE
    nc = _get_nc(mm_dtype)
    W_dev, bias = _prep_weights(coeff, scale_base, scale_spline, base_bias,
                                mm_dtype)
    xr = x.reshape(NCORES, N_PER, IN_F)
    in_maps = []
    for b in range(NCORES):
        t_b = np.ascontiguousarray((2.0 * xr[b]).T.astype(np.float32))
        in_maps.append({"t_in": t_b, "w_in": W_dev})

    trace = bool(int(os.environ.get("KERNEL_TRACE", "0")))
    res = bass_utils.run_bass_kernel_spmd(
        nc, in_maps, core_ids=list(range(NCORES)), trace=trace
    )
    global LAST_RESULT
    LAST_RESULT = res
    out = np.stack([res.results[b]["out"].T for b in range(NCORES)], axis=0)
    out = out + bias[None, None, :]
    return out.reshape(B, S, OUT_F).astype(np.float32)


LAST_RESULT = None
